# revision 24
# baseline (speedup 1.0000x reference)
"""BertMoELayer (B=4, S=2048, H=768, F=3072, E=8, top-2) on 8 Trainium2 cores.

Expert-parallel: one expert per core; the host evaluates the router in fp32
only to DECIDE the shard assignment (which tokens go to which core, matching
jax.lax.top_k tie-breaking) and gathers each core's token subset. All numeric
computation of the layer runs on device:

  per core c, over its gathered tokens (capacity = max expert load, exact):
    logitsT = WrT^T @ xT                     (bf16 matmul, fp32 psum)
    w_c     = 1 / (exp(m1-lc) + exp(m2-lc))  (smooth top-2 softmax weight;
                                              m1/m2 = top-2 of this token's
                                              logits, lc = this expert's logit)
    hT      = gelu(WiT^T @ xT + bi[c])       (bf16 matmul, fp32 psum)
    out_c   = w_c * (hT^T @ WoT + bo[c])     (bf16 matmul, fp32 psum)

The smooth w formula has no comparison cliffs: when bf16 logits reorder a
near-tie relative to the host's fp32 selection, the weight degrades
continuously (the swapped logits are equal to within the noise), so no
selection-consistency hazard exists between host and device. Logits are
bit-identical across cores (same k-chain accumulation order), so the two
selected cores' weights sum to exactly softmax's 1.

All tensors are HOST-PREPACKED into SBUF-partition-major layout ([128, ...]
with each partition's bytes contiguous in DRAM), so every DMA moves 3-9KB
contiguous lines per partition; 1KB-line views were measured at only
~150-200 GB/s per queue and starved the weight stream.

The host unshards by scatter-adding each core's (already weighted) rows.
"""

import numpy as np
import ml_dtypes

import concourse.bass as bass
import concourse.tile as tile
from concourse import bacc, mybir
from concourse.bass_utils import run_bass_kernel_spmd
from concourse.masks import make_identity

B, S, H, F, E = 4, 2048, 768, 3072, 8
T = B * S
N_CORES = 8
TOP_K = 2

P = 128          # SBUF partitions
KH = H // P      # 6   h-chunks
KF = F // P      # 24  f-chunks
HO = 384         # output free-dim split (2 x 384 = 768)

F32 = mybir.dt.float32
BF16 = mybir.dt.bfloat16
BF16_NP = ml_dtypes.bfloat16

# wi column groups (in j units of 128): small first group so the first
# L1 chain's weight DMA lands quickly at startup
WI_GROUPS = (1, 4, 4, 4, 4, 4, 3)


def make_blocks(cap: int):
    """Token blocks: small first block for fast start, 512-blocks in the
    middle, tail <= 512. All blocks except the last are multiples of 128
    (the batched out-DMA view indexes 128-row DRAM tiles by block start);
    non-tail blocks are >= 256 so L1 chains stay matmul-bound."""
    assert cap >= 512
    blocks = []
    rem = cap
    while rem > 768:
        blocks.append(512)
        rem -= 512
    if rem > 512:
        blocks.append(256)
        rem -= 256
    blocks.append(rem)
    assert sum(blocks) == cap
    assert all(b % 128 == 0 for b in blocks[:-1]) and blocks[-1] <= 512
    return blocks


def build_nc(cap: int):
    """Per-core program: router weight + dense expert FFN over `cap` tokens."""
    blocks = make_blocks(cap)
    nblk = len(blocks)
    ntile_total = sum((b + P - 1) // P for b in blocks)

    # Bacc (not plain Bass): its compile() pass splits multi-wait instructions
    # into event-semaphore chains, which walrus requires (max 1 wait per inst).
    nc = bacc.Bacc(None)

    # All inputs prepacked on host to [128 partitions, contiguous bytes].
    xg = nc.declare_dram_parameter("xg", [P, KH * cap], BF16, isOutput=False)
    wiT = nc.declare_dram_parameter("wiT", [P, KH * F], BF16, isOutput=False)
    woT = nc.declare_dram_parameter("woT", [P, KF * H], BF16, isOutput=False)
    wrT = nc.declare_dram_parameter("wrT", [P, KH * E], BF16, isOutput=False)
    bi = nc.declare_dram_parameter("bi", [P, KF], F32, isOutput=False)
    bo = nc.declare_dram_parameter("bo", [H], F32, isOutput=False)
    esel = nc.declare_dram_parameter("esel", [E], F32, isOutput=False)
    out = nc.declare_dram_parameter("out", [P, ntile_total, H], F32, isOutput=True)

    # j (0..23) -> (wi group tile index, local column slice)
    j_map = []
    for gi, gw in enumerate(WI_GROUPS):
        for jj in range(gw):
            j_map.append((gi, jj))
    # wi group g covers columns [goff[g]*128, (goff[g]+gw)*128)
    goff = [sum(WI_GROUPS[:g]) for g in range(len(WI_GROUPS))]
    # L1 chains consume j's in GROUP-ARRIVAL order: groups are streamed on
    # two DMA rings (scalar: 0,2,5; sync: 1,3,4,6) and block 0 eats them as
    # fast as they land — consuming in-order would stall on the stream.
    GROUP_ORDER = (0, 2, 1, 3, 4, 5, 6)
    j_order = [
        goff[g] + jj for g in GROUP_ORDER for jj in range(WI_GROUPS[g])
    ]

    with tile.TileContext(nc) as tc:
        with (
            tc.tile_pool(name="weights", bufs=1) as wpool,
            tc.tile_pool(name="xin", bufs=3) as xpool,
            tc.tile_pool(name="hbuf", bufs=2) as hpool,
            tc.tile_pool(name="obuf", bufs=2) as opool,
            tc.tile_pool(name="router", bufs=2) as rpool,
            tc.tile_pool(name="psum_h", bufs=3, space="PSUM") as ph_pool,
            tc.tile_pool(name="psum_o", bufs=3, space="PSUM") as po_pool,
            tc.tile_pool(name="psum_r", bufs=1, space="PSUM") as pr_pool,
            tc.tile_pool(name="psum_rt", bufs=1, space="PSUM") as prt_pool,
        ):
            # ---- preamble DMAs, split across the two HWDGE rings (sync +
            # scalar) in consumption order. xg block ib lives at flat offset
            # KH*t0 (block-major host packing -> 6KB lines / partition). ----
            def x_dma(eng, xt, t0, b):
                eng.dma_start(
                    out=xt,
                    in_=xg[:, KH * t0 : KH * (t0 + b)].rearrange(
                        "p (k t) -> p k t", k=KH
                    ),
                )

            def wig_dma(eng, wt, g):
                a = KH * P * goff[g]
                w = KH * P * WI_GROUPS[g]
                eng.dma_start(
                    out=wt,
                    in_=wiT[:, a : a + w].rearrange("p (k c) -> p k c", k=KH),
                )

            # Engine budget note: DMA_DIRECT2D injection costs ~0.7-1.9us of
            # the ISSUING engine's time. The scalar engine must be free for
            # gelus by ~12us, so it only issues the three critical early
            # tiles; everything else goes on sync (SP ring) or gpsimd (SW
            # ring). Per-ring HBM bandwidth is ~200-250 GB/s concurrent.
            x_tiles = {}
            b0 = blocks[0]
            x0_bf = xpool.tile([P, KH, b0], BF16, tag="xb", name="x0_bf")
            x_tiles[0] = x0_bf

            wrT_sb = wpool.tile([P, KH, E], BF16)
            nc.sync.dma_start(
                out=wrT_sb, in_=wrT.rearrange("p (k e) -> p k e", k=KH)
            )
            wi_groups = [
                wpool.tile(
                    [P, KH, gw * P], BF16, tag=f"wig{gi}", name=f"wig{gi}"
                )
                for gi, gw in enumerate(WI_GROUPS)
            ]
            bi_sb = wpool.tile([P, KF], F32)
            woT_sb = wpool.tile([P, KF, H], BF16)

            # scalar ring: first wi group + bias + back half of x0
            wig_dma(nc.scalar, wi_groups[0], 0)
            nc.scalar.dma_start(out=bi_sb, in_=bi[:, :])
            half = (KH // 2) * b0
            nc.sync.dma_start(
                out=x0_bf[:, 0 : KH // 2, :],
                in_=xg[:, 0:half].rearrange("p (k t) -> p k t", k=KH // 2),
            )
            nc.scalar.dma_start(
                out=x0_bf[:, KH // 2 : KH, :],
                in_=xg[:, half : KH * b0].rearrange("p (k t) -> p k t", k=KH // 2),
            )
            wig_dma(nc.scalar, wi_groups[2], 2)
            wig_dma(nc.scalar, wi_groups[5], 5)
            # sync ring: remaining wi groups in need order, then woT front half
            wig_dma(nc.sync, wi_groups[1], 1)
            wig_dma(nc.sync, wi_groups[3], 3)
            wig_dma(nc.sync, wi_groups[4], 4)
            wig_dma(nc.sync, wi_groups[6], 6)

            def wo_dma(eng, g, n):
                eng.dma_start(
                    out=woT_sb[:, g : g + n, :],
                    in_=woT[:, g * H : (g + n) * H].rearrange(
                        "p (j h) -> p j h", j=n
                    ),
                )

            # one-hot expert selector, broadcast to all partitions (before bo:
            # the router needs it ~15us in, bo only at the first L2 epilogue)
            esel_sb = wpool.tile([P, E], F32)
            nc.gpsimd.dma_start(out=esel_sb, in_=esel[None, :].to_broadcast([P, E]))
            wo_dma(nc.sync, 0, 6)
            wo_dma(nc.sync, 6, 6)
            wo_dma(nc.gpsimd, 12, 6)
            wo_dma(nc.gpsimd, 18, 6)
            # bo broadcast to all 128 partitions (it is added along the free dim)
            bo_sb = wpool.tile([P, H], F32)
            nc.gpsimd.dma_start(out=bo_sb, in_=bo[None, :].to_broadcast([P, H]))
            # identity for the PE-mode transpose of the router logits
            id8 = wpool.tile([E, E], F32, name="id8")
            make_identity(nc, id8)

            def router_logits(x_bf, b):
                # logitsT [E, b] via the same bf16 x the FFN uses; fp32 psum.
                pslT = pr_pool.tile([E, b], F32, tag="pr")
                for k in range(KH):
                    nc.tensor.matmul(
                        pslT,
                        lhsT=wrT_sb[:, k, :],
                        rhs=x_bf[:, k, :],
                        start=(k == 0),
                        stop=(k == KH - 1),
                    )
                # psum -> sbuf copy on the SCALAR engine: the DVE queue is
                # in-order and its tail (previous block's L2 epilogue ops,
                # which wait on L2 psums) would stall this copy and with it
                # the next block's transposes on the PE.
                lgT_sb = rpool.tile([E, b], F32, tag="lgT")
                nc.scalar.activation(
                    lgT_sb, pslT, mybir.ActivationFunctionType.Copy
                )
                return lgT_sb

            def router_chain(ts, ts0, tsz, lgT_sb, pst_blk, aa_blk):
                # transpose this ts's logits back to [t, e] into a shared psum
                # tile. Only ts==0 uses start=True: it marks the whole 2KB
                # psum zero-region pending-zero; later transposes zero their
                # own bytes on first touch without wiping earlier columns.
                nc.tensor.matmul(
                    pst_blk[0:tsz, E * ts : E * (ts + 1)],
                    lhsT=lgT_sb[:, ts0 : ts0 + tsz],
                    rhs=id8,
                    is_transpose=True,
                    start=(ts == 0),
                    stop=True,
                    skip_group_check=True,
                )
                lg = pst_blk[0:tsz, E * ts : E * (ts + 1)]
                # top-2: m1 = max, m2 = max with the argmax masked out
                m1 = rpool.tile([P, 1], F32, tag="m1")
                nc.vector.reduce_max(m1[0:tsz], lg, axis=mybir.AxisListType.X)
                ge = rpool.tile([P, E], F32, tag="ge")
                nc.vector.tensor_scalar(
                    ge[0:tsz], lg, scalar1=m1[0:tsz], scalar2=-1e30,
                    op0=mybir.AluOpType.is_ge, op1=mybir.AluOpType.mult,
                )
                mk = rpool.tile([P, E], F32, tag="mk")
                nc.vector.tensor_tensor(mk[0:tsz], lg, ge[0:tsz], op=mybir.AluOpType.add)
                m2 = rpool.tile([P, 1], F32, tag="m2")
                nc.vector.reduce_max(m2[0:tsz], mk[0:tsz], axis=mybir.AxisListType.X)
                # this core's logit: lc = sum(lg * esel)
                lce = rpool.tile([P, E], F32, tag="lce")
                nc.vector.tensor_tensor(
                    lce[0:tsz], lg, esel_sb[0:tsz], op=mybir.AluOpType.mult
                )
                lc = rpool.tile([P, 1], F32, tag="lc")
                nc.vector.reduce_sum(lc[0:tsz], lce[0:tsz], axis=mybir.AxisListType.X)
                # w = 1 / (exp(m1-lc) + exp(m2-lc)); lc is m1 or m2 up to
                # rounding, so both args are in [-eps, m1-m2]: no overflow.
                # Only the exp ARGS are computed here; the exp itself is
                # batched once per block (after the gelus) so the scalar
                # engine swaps activation tables at most twice per block,
                # during the gelu-free L2 window.
                nc.vector.tensor_tensor(
                    aa_blk[0:tsz, 2 * ts : 2 * ts + 1], m1[0:tsz], lc[0:tsz],
                    op=mybir.AluOpType.subtract,
                )
                nc.vector.tensor_tensor(
                    aa_blk[0:tsz, 2 * ts + 1 : 2 * ts + 2], m2[0:tsz], lc[0:tsz],
                    op=mybir.AluOpType.subtract,
                )

            t0 = 0
            n0 = 0
            for ib, b in enumerate(blocks):
                # ts tiles within the block (last may be partial)
                ts_sizes = [P] * (b // P) + ([b % P] if b % P else [])
                ntiles = len(ts_sizes)
                last_blk = ib == nblk - 1

                x_bf = x_tiles.pop(ib)
                # prefetch next block's x; issued here (not in the preamble) so
                # it doesn't compete with the wi/wo weight stream at startup
                if ib + 1 < nblk:
                    bn = blocks[ib + 1]
                    x_next = xpool.tile([P, KH, bn], BF16, tag="xb", name="x_next")
                    x_tiles[ib + 1] = x_next
                    x_dma(nc.sync, x_next, t0 + b, bn)

                w_blk = rpool.tile([P, ntiles], F32, tag="w")
                aa_blk = rpool.tile([P, 2 * ntiles], F32, tag="aa")
                pst_blk = prt_pool.tile([P, E * ntiles], F32, tag="prt")

                # ---- layer 1: hT[f, t] = gelu(WiT^T @ xT + bi), with the
                # router work interleaved between the dense j-chains so the
                # PE activity stays dense. Router logits go right after the
                # j=0 chain: the extra PE time buys slack for the wi-group
                # weight stream at startup. ----
                hT = hpool.tile([P, KF, b], BF16, tag="hT")
                for idx, j in enumerate(j_order):
                    gi, jj = j_map[j]
                    ps = ph_pool.tile([P, b], F32, tag="ph")
                    wig = wi_groups[gi]
                    for k in range(KH):
                        nc.tensor.matmul(
                            ps,
                            lhsT=wig[:, k, jj * P : (jj + 1) * P],
                            rhs=x_bf[:, k, :],
                            start=(k == 0),
                            stop=(k == KH - 1),
                        )
                    nc.scalar.activation(
                        out=hT[:, j, :],
                        in_=ps,
                        func=mybir.ActivationFunctionType.Gelu,
                        bias=bi_sb[:, j : j + 1],
                        scale=1.0,
                    )
                    if idx == 0:
                        lgT_sb = router_logits(x_bf, b)
                    elif 1 <= idx < 1 + ntiles:
                        ts = idx - 1
                        router_chain(
                            ts, ts * P, ts_sizes[ts], lgT_sb, pst_blk, aa_blk
                        )

                # batched exp + combine: one table swap pair per block
                ee_blk = rpool.tile([P, 2 * ntiles], F32, tag="ee")
                nc.scalar.activation(
                    ee_blk, aa_blk, mybir.ActivationFunctionType.Exp
                )
                den_blk = rpool.tile([P, ntiles], F32, tag="den")
                nc.vector.tensor_tensor(
                    den_blk,
                    ee_blk[:, 0 : 2 * ntiles : 2],
                    ee_blk[:, 1 : 2 * ntiles : 2],
                    op=mybir.AluOpType.add,
                )
                nc.vector.reciprocal(w_blk, den_blk)

                # ---- layer 2 + bo + routing-weight scale ----
                o_blk = opool.tile([P, ntiles, H], F32, tag="os")
                for ts in range(ntiles):
                    tsz = ts_sizes[ts]
                    po_a = po_pool.tile([P, HO], F32, tag="po")
                    po_b = po_pool.tile([P, HO], F32, tag="po")
                    for j in range(KF):
                        lhsT = hT[:, j, ts * P : ts * P + tsz]
                        nc.tensor.matmul(
                            po_a[0:tsz], lhsT=lhsT, rhs=woT_sb[:, j, 0:HO],
                            start=(j == 0), stop=(j == KF - 1),
                        )
                        nc.tensor.matmul(
                            po_b[0:tsz], lhsT=lhsT, rhs=woT_sb[:, j, HO : 2 * HO],
                            start=(j == 0), stop=(j == KF - 1),
                        )
                    o_sl = o_blk[0:tsz, ts, :]
                    wcol = w_blk[0:tsz, ts : ts + 1]
                    nc.vector.tensor_tensor(
                        o_sl[:, 0:HO], po_a[0:tsz], bo_sb[0:tsz, 0:HO],
                        op=mybir.AluOpType.add,
                    )
                    nc.vector.tensor_tensor(
                        o_sl[:, HO : 2 * HO], po_b[0:tsz], bo_sb[0:tsz, HO : 2 * HO],
                        op=mybir.AluOpType.add,
                    )
                    nc.vector.tensor_scalar_mul(o_sl, o_sl, scalar1=wcol)
                    if last_blk:
                        # per-tile writes on the last block: the final DMA
                        # after the last epilogue is then one small tile, not
                        # the whole block (shorter teardown tail)
                        nc.sync.dma_start(
                            out=out[0:tsz, n0 + ts, :], in_=o_sl
                        )

                if not last_blk:
                    # one batched out DMA per block (all tiles full here)
                    nc.sync.dma_start(
                        out=out[:, n0 : n0 + ntiles, :], in_=o_blk
                    )
                n0 += ntiles
                t0 += b

    nc.compile()
    return nc


_NC_CACHE: dict = {}


def _get_nc(cap: int):
    if cap not in _NC_CACHE:
        _NC_CACHE[cap] = build_nc(cap)
    return _NC_CACHE[cap]


def _ensure_axon_hooks_module():
    """run_bass_kernel_spmd(trace=True) (e.g. via env BASS_TRACE=1) imports
    antenv.axon_hooks, which some images lack even though the boot code that
    would register the NTFF hook is present. Provide the module and register
    the real hook when available so tracing works instead of crashing."""
    try:
        import antenv.axon_hooks  # noqa: F401

        return
    except ImportError:
        pass
    try:
        import sys
        import types

        import antenv  # noqa: F401

        mod = types.ModuleType("antenv.axon_hooks")
        state = {"hook": None}
        mod.set_axon_ntff_profile_hook = lambda h: state.__setitem__("hook", h)
        mod.get_axon_ntff_profile_hook = lambda: state["hook"]
        try:
            from trn_agent_boot.trn_boot import _ntff_profile_via_ctypes

            mod.set_axon_ntff_profile_hook(
                _ntff_profile_via_ctypes("/opt/axon/libaxon_pjrt.so")
            )
        except Exception:
            pass
        sys.modules["antenv.axon_hooks"] = mod
    except Exception:
        pass


def _shard_tokens(xf, Wr):
    """Host-side sharding function: top-2 expert index per token (matches
    jax.lax.top_k tie-breaking: lowest index wins on ties)."""
    logits = xf.astype(np.float32) @ np.asarray(Wr, np.float32).T  # [T, E]
    i1 = np.argmax(logits, axis=1)
    l2 = logits.copy()
    l2[np.arange(len(i1)), i1] = -np.inf
    i2 = np.argmax(l2, axis=1)
    tokens = np.arange(logits.shape[0])
    tok_lists = []
    for c in range(N_CORES):
        tok_lists.append(np.concatenate([tokens[i1 == c], tokens[i2 == c]]))
    return tok_lists


def _pack_kpf(a2d, k):
    """[k*128, N] row-major -> [128, k*N] partition-major (k-major per row)."""
    kk, n = a2d.shape
    assert kk == k * P
    return np.ascontiguousarray(
        a2d.reshape(k, P, n).transpose(1, 0, 2).reshape(P, k * n)
    )


def _pack_wi_groups(wiT2d):
    """[H, F] -> [128, KH*F] GROUP-major: each wi column group's
    [KH, group_cols] block is contiguous per partition."""
    v = wiT2d.reshape(KH, P, F)
    parts = []
    c0 = 0
    for gw in WI_GROUPS:
        parts.append(
            v[:, :, c0 : c0 + gw * P].transpose(1, 0, 2).reshape(P, KH * gw * P)
        )
        c0 += gw * P
    return np.ascontiguousarray(np.concatenate(parts, axis=1))


def kernel(x, Wr, Wi, bi, Wo, bo, _trace=False):
    x = np.asarray(x)
    xf = x.reshape(-1, H).astype(np.float32)
    tok_lists = _shard_tokens(xf, Wr)
    cap = max(512, max(len(tl) for tl in tok_lists))
    blocks = make_blocks(cap)

    xT = np.ascontiguousarray(xf.T).astype(BF16_NP)  # [H, T] bf16
    wrT_p = _pack_kpf(
        np.ascontiguousarray(np.asarray(Wr, np.float32).T).astype(BF16_NP), KH
    )
    bi_full = np.asarray(bi, np.float32)
    bo_full = np.asarray(bo, np.float32)

    in_maps = []
    for c in range(N_CORES):
        tl = tok_lists[c]
        xg = np.zeros((H, cap), dtype=BF16_NP)
        xg[:, : len(tl)] = xT[:, tl]
        # block-major packing: [128, sum_b KH*b], block ib at offset KH*t0
        xg_k = xg.reshape(KH, P, cap)
        xg_p = np.empty((P, KH * cap), dtype=BF16_NP)
        t0 = 0
        for b in blocks:
            xg_p[:, KH * t0 : KH * (t0 + b)] = (
                xg_k[:, :, t0 : t0 + b].transpose(1, 0, 2).reshape(P, KH * b)
            )
            t0 += b
        sel = np.zeros(E, np.float32)
        sel[c] = 1.0
        in_maps.append(
            {
                "xg": xg_p,
                "wiT": _pack_wi_groups(
                    np.asarray(Wi[c], np.float32).T.astype(BF16_NP)
                ),
                "woT": _pack_kpf(
                    np.ascontiguousarray(np.asarray(Wo[c], np.float32).T).astype(
                        BF16_NP
                    ),
                    KF,
                ),
                "wrT": wrT_p,
                "bi": _pack_kpf(bi_full[c].reshape(F, 1), KF).reshape(P, KF),
                "bo": bo_full[c],
                "esel": sel,
            }
        )

    _ensure_axon_hooks_module()
    nc = _get_nc(cap)
    res = run_bass_kernel_spmd(
        nc, in_maps, core_ids=list(range(N_CORES)), trace=_trace
    )

    # Unshard: scatter-add the per-expert (already routing-weighted) rows.
    out = np.zeros((T, H), dtype=np.float32)
    for c in range(N_CORES):
        tl = tok_lists[c]
        # out param is [128, ntile_total, H]: token t0+ts*128+p -> [p, n0+ts, :]
        o = res.results[c]["out"]  # [P, NT, H]
        o_rows = o.transpose(1, 0, 2).reshape(-1, H)[: len(tl)]
        out[tl] += o_rows
    out = out.reshape(x.shape)
    if _trace:
        return out, res
    return out


# revision 28
# speedup vs baseline: 1.0109x; 1.0109x over previous
"""BertMoELayer (B=4, S=2048, H=768, F=3072, E=8, top-2) on 8 Trainium2 cores.

Expert-parallel: one expert per core; the host evaluates the router in fp32
only to DECIDE the shard assignment (which tokens go to which core, matching
jax.lax.top_k tie-breaking) and gathers each core's token subset. All numeric
computation of the layer runs on device:

  per core c, over its gathered tokens (capacity = max expert load, exact):
    logitsT = WrT^T @ xT                     (bf16 matmul, fp32 psum)
    w_c     = 1 / (exp(m1-lc) + exp(m2-lc))  (smooth top-2 softmax weight;
                                              m1/m2 = top-2 of this token's
                                              logits, lc = this expert's logit)
    hT      = gelu(WiT^T @ xT + bi[c])       (bf16 matmul, fp32 psum)
    out_c   = w_c * (hT^T @ WoT + bo[c])     (bf16 matmul, fp32 psum)

The smooth w formula has no comparison cliffs: when bf16 logits reorder a
near-tie relative to the host's fp32 selection, the weight degrades
continuously (the swapped logits are equal to within the noise), so no
selection-consistency hazard exists between host and device. Logits are
bit-identical across cores (same k-chain accumulation order), so the two
selected cores' weights sum to exactly softmax's 1.

All tensors are HOST-PREPACKED into SBUF-partition-major layout ([128, ...]
with each partition's bytes contiguous in DRAM), so every DMA moves 3-9KB
contiguous lines per partition; 1KB-line views were measured at only
~150-200 GB/s per queue and starved the weight stream.

The host unshards by scatter-adding each core's (already weighted) rows.
"""

import numpy as np
import ml_dtypes

import concourse.bass as bass
import concourse.tile as tile
from concourse import bacc, mybir
from concourse.bass_utils import run_bass_kernel_spmd
from concourse.masks import make_identity

B, S, H, F, E = 4, 2048, 768, 3072, 8
T = B * S
N_CORES = 8
TOP_K = 2

P = 128          # SBUF partitions
KH = H // P      # 6   h-chunks
KF = F // P      # 24  f-chunks
HO = 384         # output free-dim split (2 x 384 = 768)

F32 = mybir.dt.float32
BF16 = mybir.dt.bfloat16
BF16_NP = ml_dtypes.bfloat16

# wi column groups (in j units of 128): small first group so the first
# L1 chain's weight DMA lands quickly at startup
WI_GROUPS = (1, 4, 4, 4, 4, 4, 3)


def make_blocks(cap: int):
    """Token blocks: small first block for fast start, 512-blocks in the
    middle, tail <= 512. All blocks except the last are multiples of 128
    (the batched out-DMA view indexes 128-row DRAM tiles by block start);
    non-tail blocks are >= 256 so L1 chains stay matmul-bound."""
    assert cap >= 512
    blocks = []
    rem = cap
    while rem > 768:
        blocks.append(512)
        rem -= 512
    if rem > 512:
        blocks.append(256)
        rem -= 256
    blocks.append(rem)
    assert sum(blocks) == cap
    assert all(b % 128 == 0 for b in blocks[:-1]) and blocks[-1] <= 512
    return blocks


def build_nc(cap: int):
    """Per-core program: router weight + dense expert FFN over `cap` tokens."""
    blocks = make_blocks(cap)
    nblk = len(blocks)
    ntile_total = sum((b + P - 1) // P for b in blocks)

    # Bacc (not plain Bass): its compile() pass splits multi-wait instructions
    # into event-semaphore chains, which walrus requires (max 1 wait per inst).
    nc = bacc.Bacc(None)

    # All inputs prepacked on host to [128 partitions, contiguous bytes].
    xg = nc.declare_dram_parameter("xg", [P, KH * cap], BF16, isOutput=False)
    wiT = nc.declare_dram_parameter("wiT", [P, KH * F], BF16, isOutput=False)
    woT = nc.declare_dram_parameter("woT", [P, KF * H], BF16, isOutput=False)
    wrT = nc.declare_dram_parameter("wrT", [P, KH * E], BF16, isOutput=False)
    bi = nc.declare_dram_parameter("bi", [P, KF], F32, isOutput=False)
    bo = nc.declare_dram_parameter("bo", [H], F32, isOutput=False)
    esel = nc.declare_dram_parameter("esel", [E], F32, isOutput=False)
    out = nc.declare_dram_parameter("out", [P, ntile_total, H], F32, isOutput=True)

    # j (0..23) -> (wi group tile index, local column slice)
    j_map = []
    for gi, gw in enumerate(WI_GROUPS):
        for jj in range(gw):
            j_map.append((gi, jj))
    # wi group g covers columns [goff[g]*128, (goff[g]+gw)*128)
    goff = [sum(WI_GROUPS[:g]) for g in range(len(WI_GROUPS))]
    # L1 chains consume j's in GROUP-ARRIVAL order; with the current ring
    # assignment the groups arrive in natural order.
    GROUP_ORDER = (0, 1, 2, 3, 4, 5, 6)
    j_order = [
        goff[g] + jj for g in GROUP_ORDER for jj in range(WI_GROUPS[g])
    ]

    with tile.TileContext(nc) as tc:
        with (
            tc.tile_pool(name="weights", bufs=1) as wpool,
            tc.tile_pool(name="xin", bufs=3) as xpool,
            tc.tile_pool(name="hbuf", bufs=2) as hpool,
            tc.tile_pool(name="obuf", bufs=2) as opool,
            tc.tile_pool(name="router", bufs=2) as rpool,
            tc.tile_pool(name="psum_h", bufs=4, space="PSUM") as ph_pool,
            tc.tile_pool(name="psum_o", bufs=2, space="PSUM") as po_pool,
            tc.tile_pool(name="psum_r", bufs=1, space="PSUM") as pr_pool,
            tc.tile_pool(name="psum_rt", bufs=1, space="PSUM") as prt_pool,
        ):
            # ---- preamble DMAs, split across the two HWDGE rings (sync +
            # scalar) in consumption order. xg block ib lives at flat offset
            # KH*t0 (block-major host packing -> 6KB lines / partition). ----
            def x_dma(eng, xt, t0, b):
                eng.dma_start(
                    out=xt,
                    in_=xg[:, KH * t0 : KH * (t0 + b)].rearrange(
                        "p (k t) -> p k t", k=KH
                    ),
                )

            def wig_dma(eng, wt, g):
                a = KH * P * goff[g]
                w = KH * P * WI_GROUPS[g]
                eng.dma_start(
                    out=wt,
                    in_=wiT[:, a : a + w].rearrange("p (k c) -> p k c", k=KH),
                )

            # Engine budget note: DMA_DIRECT2D injection costs ~0.7-1.9us of
            # the ISSUING engine's time. The scalar engine must be free for
            # gelus by ~12us, so it only issues the three critical early
            # tiles; everything else goes on sync (SP ring) or gpsimd (SW
            # ring). Per-ring HBM bandwidth is ~200-250 GB/s concurrent.
            x_tiles = {}
            b0 = blocks[0]
            x0_bf = xpool.tile([P, KH, b0], BF16, tag="xb", name="x0_bf")
            x_tiles[0] = x0_bf

            wrT_sb = wpool.tile([P, KH, E], BF16)
            nc.sync.dma_start(
                out=wrT_sb, in_=wrT.rearrange("p (k e) -> p k e", k=KH)
            )
            wi_groups = [
                wpool.tile(
                    [P, KH, gw * P], BF16, tag=f"wig{gi}", name=f"wig{gi}"
                )
                for gi, gw in enumerate(WI_GROUPS)
            ]
            bi_sb = wpool.tile([P, KF], F32)
            woT_sb = wpool.tile([P, KF, H], BF16)

            def wo_dma(eng, g, n):
                eng.dma_start(
                    out=woT_sb[:, g : g + n, :],
                    in_=woT[:, g * H : (g + n) * H].rearrange(
                        "p (j h) -> p j h", j=n
                    ),
                )

            # scalar ring: wi group 0 + back half of x0 + bias + wi group 2.
            # Nothing else early: every DMA_DIRECT2D costs ~0.7-1.9us of the
            # issuing engine's time and gelus need the scalar engine by ~12us.
            wig_dma(nc.scalar, wi_groups[0], 0)
            half = (KH // 2) * b0
            nc.sync.dma_start(
                out=x0_bf[:, 0 : KH // 2, :],
                in_=xg[:, 0:half].rearrange("p (k t) -> p k t", k=KH // 2),
            )
            nc.scalar.dma_start(
                out=x0_bf[:, KH // 2 : KH, :],
                in_=xg[:, half : KH * b0].rearrange("p (k t) -> p k t", k=KH // 2),
            )
            nc.scalar.dma_start(out=bi_sb, in_=bi[:, :])
            wig_dma(nc.scalar, wi_groups[2], 2)
            # sync ring: wi groups 1,3,6 then the front woT half and x1;
            # gpsimd (SW ring): esel, wi group 4, bo. wig5/woT2/woT3 are
            # paced from inside block 0's j-loop (emitted between gelus) so
            # they don't compete for HBM during the startup crunch.
            wig_dma(nc.sync, wi_groups[1], 1)
            wig_dma(nc.sync, wi_groups[3], 3)
            wig_dma(nc.sync, wi_groups[6], 6)
            esel_sb = wpool.tile([P, E], F32)
            nc.gpsimd.dma_start(out=esel_sb, in_=esel[None, :].to_broadcast([P, E]))
            wig_dma(nc.gpsimd, wi_groups[4], 4)
            wo_dma(nc.sync, 0, 6)
            wo_dma(nc.sync, 6, 6)
            bo_sb = wpool.tile([P, H], F32)
            nc.gpsimd.dma_start(out=bo_sb, in_=bo[None, :].to_broadcast([P, H]))
            # identity for the PE-mode transpose of the router logits
            id8 = wpool.tile([E, E], F32, name="id8")
            make_identity(nc, id8)

            def router_logits(x_bf, b):
                # logitsT [E, b] via the same bf16 x the FFN uses; fp32 psum.
                pslT = pr_pool.tile([E, b], F32, tag="pr")
                for k in range(KH):
                    nc.tensor.matmul(
                        pslT,
                        lhsT=wrT_sb[:, k, :],
                        rhs=x_bf[:, k, :],
                        start=(k == 0),
                        stop=(k == KH - 1),
                    )
                # psum -> sbuf copy on the SCALAR engine: the DVE queue is
                # in-order and its tail (previous block's L2 epilogue ops,
                # which wait on L2 psums) would stall this copy and with it
                # the next block's transposes on the PE.
                lgT_sb = rpool.tile([E, b], F32, tag="lgT")
                nc.scalar.activation(
                    lgT_sb, pslT, mybir.ActivationFunctionType.Copy
                )
                return lgT_sb

            def router_chain(ts, ts0, tsz, lgT_sb, pst_blk, aa_blk):
                # transpose this ts's logits back to [t, e] into a shared psum
                # tile. Only ts==0 uses start=True: it marks the whole 2KB
                # psum zero-region pending-zero; later transposes zero their
                # own bytes on first touch without wiping earlier columns.
                nc.tensor.matmul(
                    pst_blk[0:tsz, E * ts : E * (ts + 1)],
                    lhsT=lgT_sb[:, ts0 : ts0 + tsz],
                    rhs=id8,
                    is_transpose=True,
                    start=(ts == 0),
                    stop=True,
                    skip_group_check=True,
                )
                lg = pst_blk[0:tsz, E * ts : E * (ts + 1)]
                # top-2: m1 = max, m2 = max with the argmax masked out
                m1 = rpool.tile([P, 1], F32, tag="m1")
                nc.vector.reduce_max(m1[0:tsz], lg, axis=mybir.AxisListType.X)
                ge = rpool.tile([P, E], F32, tag="ge")
                nc.vector.tensor_scalar(
                    ge[0:tsz], lg, scalar1=m1[0:tsz], scalar2=-1e30,
                    op0=mybir.AluOpType.is_ge, op1=mybir.AluOpType.mult,
                )
                mk = rpool.tile([P, E], F32, tag="mk")
                nc.vector.tensor_tensor(mk[0:tsz], lg, ge[0:tsz], op=mybir.AluOpType.add)
                m2 = rpool.tile([P, 1], F32, tag="m2")
                nc.vector.reduce_max(m2[0:tsz], mk[0:tsz], axis=mybir.AxisListType.X)
                # this core's logit: lc = sum(lg * esel)
                lce = rpool.tile([P, E], F32, tag="lce")
                nc.vector.tensor_tensor(
                    lce[0:tsz], lg, esel_sb[0:tsz], op=mybir.AluOpType.mult
                )
                lc = rpool.tile([P, 1], F32, tag="lc")
                nc.vector.reduce_sum(lc[0:tsz], lce[0:tsz], axis=mybir.AxisListType.X)
                # w = 1 / (exp(m1-lc) + exp(m2-lc)); lc is m1 or m2 up to
                # rounding, so both args are in [-eps, m1-m2]: no overflow.
                # Only the exp ARGS are computed here; the exp itself is
                # batched once per block (after the gelus) so the scalar
                # engine swaps activation tables at most twice per block,
                # during the gelu-free L2 window.
                nc.vector.tensor_tensor(
                    aa_blk[0:tsz, 2 * ts : 2 * ts + 1], m1[0:tsz], lc[0:tsz],
                    op=mybir.AluOpType.subtract,
                )
                nc.vector.tensor_tensor(
                    aa_blk[0:tsz, 2 * ts + 1 : 2 * ts + 2], m2[0:tsz], lc[0:tsz],
                    op=mybir.AluOpType.subtract,
                )

            t0 = 0
            n0 = 0
            for ib, b in enumerate(blocks):
                # ts tiles within the block (last may be partial)
                ts_sizes = [P] * (b // P) + ([b % P] if b % P else [])
                ntiles = len(ts_sizes)
                last_blk = ib == nblk - 1

                x_bf = x_tiles.pop(ib)
                # prefetch next block's x; issued here (not in the preamble) so
                # it doesn't compete with the wi/wo weight stream at startup
                if ib + 1 < nblk:
                    bn = blocks[ib + 1]
                    x_next = xpool.tile([P, KH, bn], BF16, tag="xb", name="x_next")
                    x_tiles[ib + 1] = x_next
                    x_dma(nc.sync, x_next, t0 + b, bn)

                w_blk = rpool.tile([P, ntiles], F32, tag="w")
                aa_blk = rpool.tile([P, 2 * ntiles], F32, tag="aa")
                pst_blk = prt_pool.tile([P, E * ntiles], F32, tag="prt")

                # ---- layer 1: hT[f, t] = gelu(WiT^T @ xT + bi), with the
                # router work interleaved between the dense j-chains so the
                # PE activity stays dense. Router logits go right after the
                # j=0 chain: the extra PE time buys slack for the wi-group
                # weight stream at startup. ----
                hT = hpool.tile([P, KF, b], BF16, tag="hT")
                for idx, j in enumerate(j_order):
                    gi, jj = j_map[j]
                    ps = ph_pool.tile([P, b], F32, tag="ph")
                    wig = wi_groups[gi]
                    for k in range(KH):
                        nc.tensor.matmul(
                            ps,
                            lhsT=wig[:, k, jj * P : (jj + 1) * P],
                            rhs=x_bf[:, k, :],
                            start=(k == 0),
                            stop=(k == KH - 1),
                        )
                    nc.scalar.activation(
                        out=hT[:, j, :],
                        in_=ps,
                        func=mybir.ActivationFunctionType.Gelu,
                        bias=bi_sb[:, j : j + 1],
                        scale=1.0,
                    )
                    if idx == 0:
                        lgT_sb = router_logits(x_bf, b)
                    elif 1 <= idx < 1 + ntiles:
                        ts = idx - 1
                        router_chain(
                            ts, ts * P, ts_sizes[ts], lgT_sb, pst_blk, aa_blk
                        )
                    if ib == 0:
                        # paced late loads: injected between gelus so their
                        # HBM traffic starts after the startup crunch
                        if idx == 4:
                            wig_dma(nc.scalar, wi_groups[5], 5)
                        elif idx == 8:
                            wo_dma(nc.scalar, 12, 6)
                        elif idx == 12:
                            wo_dma(nc.scalar, 18, 6)

                # batched exp + combine: one table swap pair per block
                ee_blk = rpool.tile([P, 2 * ntiles], F32, tag="ee")
                nc.scalar.activation(
                    ee_blk, aa_blk, mybir.ActivationFunctionType.Exp
                )
                den_blk = rpool.tile([P, ntiles], F32, tag="den")
                nc.vector.tensor_tensor(
                    den_blk,
                    ee_blk[:, 0 : 2 * ntiles : 2],
                    ee_blk[:, 1 : 2 * ntiles : 2],
                    op=mybir.AluOpType.add,
                )
                nc.vector.reciprocal(w_blk, den_blk)

                # ---- layer 2 + bo + routing-weight scale ----
                o_blk = opool.tile([P, ntiles, H], F32, tag="os")
                for ts in range(ntiles):
                    tsz = ts_sizes[ts]
                    po_a = po_pool.tile([P, HO], F32, tag="po")
                    po_b = po_pool.tile([P, HO], F32, tag="po")
                    for j in range(KF):
                        lhsT = hT[:, j, ts * P : ts * P + tsz]
                        nc.tensor.matmul(
                            po_a[0:tsz], lhsT=lhsT, rhs=woT_sb[:, j, 0:HO],
                            start=(j == 0), stop=(j == KF - 1),
                        )
                        nc.tensor.matmul(
                            po_b[0:tsz], lhsT=lhsT, rhs=woT_sb[:, j, HO : 2 * HO],
                            start=(j == 0), stop=(j == KF - 1),
                        )
                    o_sl = o_blk[0:tsz, ts, :]
                    wcol = w_blk[0:tsz, ts : ts + 1]
                    nc.vector.tensor_tensor(
                        o_sl[:, 0:HO], po_a[0:tsz], bo_sb[0:tsz, 0:HO],
                        op=mybir.AluOpType.add,
                    )
                    nc.vector.tensor_tensor(
                        o_sl[:, HO : 2 * HO], po_b[0:tsz], bo_sb[0:tsz, HO : 2 * HO],
                        op=mybir.AluOpType.add,
                    )
                    nc.vector.tensor_scalar_mul(o_sl, o_sl, scalar1=wcol)
                    if last_blk:
                        # per-tile writes on the last block: the final DMA
                        # after the last epilogue is then one small tile, not
                        # the whole block (shorter teardown tail)
                        nc.sync.dma_start(
                            out=out[0:tsz, n0 + ts, :], in_=o_sl
                        )

                if not last_blk:
                    # one batched out DMA per block (all tiles full here)
                    nc.sync.dma_start(
                        out=out[:, n0 : n0 + ntiles, :], in_=o_blk
                    )
                n0 += ntiles
                t0 += b

    nc.compile()
    return nc


_NC_CACHE: dict = {}


def _get_nc(cap: int):
    if cap not in _NC_CACHE:
        _NC_CACHE[cap] = build_nc(cap)
    return _NC_CACHE[cap]


def _ensure_axon_hooks_module():
    """run_bass_kernel_spmd(trace=True) (e.g. via env BASS_TRACE=1) imports
    antenv.axon_hooks, which some images lack even though the boot code that
    would register the NTFF hook is present. Provide the module and register
    the real hook when available so tracing works instead of crashing."""
    try:
        import antenv.axon_hooks  # noqa: F401

        return
    except ImportError:
        pass
    try:
        import sys
        import types

        import antenv  # noqa: F401

        mod = types.ModuleType("antenv.axon_hooks")
        state = {"hook": None}
        mod.set_axon_ntff_profile_hook = lambda h: state.__setitem__("hook", h)
        mod.get_axon_ntff_profile_hook = lambda: state["hook"]
        try:
            from trn_agent_boot.trn_boot import _ntff_profile_via_ctypes

            mod.set_axon_ntff_profile_hook(
                _ntff_profile_via_ctypes("/opt/axon/libaxon_pjrt.so")
            )
        except Exception:
            pass
        sys.modules["antenv.axon_hooks"] = mod
    except Exception:
        pass


def _shard_tokens(xf, Wr):
    """Host-side sharding function: top-2 expert index per token (matches
    jax.lax.top_k tie-breaking: lowest index wins on ties)."""
    logits = xf.astype(np.float32) @ np.asarray(Wr, np.float32).T  # [T, E]
    i1 = np.argmax(logits, axis=1)
    l2 = logits.copy()
    l2[np.arange(len(i1)), i1] = -np.inf
    i2 = np.argmax(l2, axis=1)
    tokens = np.arange(logits.shape[0])
    tok_lists = []
    for c in range(N_CORES):
        tok_lists.append(np.concatenate([tokens[i1 == c], tokens[i2 == c]]))
    return tok_lists


def _pack_kpf(a2d, k):
    """[k*128, N] row-major -> [128, k*N] partition-major (k-major per row)."""
    kk, n = a2d.shape
    assert kk == k * P
    return np.ascontiguousarray(
        a2d.reshape(k, P, n).transpose(1, 0, 2).reshape(P, k * n)
    )


def _pack_wi_groups(wiT2d):
    """[H, F] -> [128, KH*F] GROUP-major: each wi column group's
    [KH, group_cols] block is contiguous per partition."""
    v = wiT2d.reshape(KH, P, F)
    parts = []
    c0 = 0
    for gw in WI_GROUPS:
        parts.append(
            v[:, :, c0 : c0 + gw * P].transpose(1, 0, 2).reshape(P, KH * gw * P)
        )
        c0 += gw * P
    return np.ascontiguousarray(np.concatenate(parts, axis=1))


def kernel(x, Wr, Wi, bi, Wo, bo, _trace=False):
    x = np.asarray(x)
    xf = x.reshape(-1, H).astype(np.float32)
    tok_lists = _shard_tokens(xf, Wr)
    cap = max(512, max(len(tl) for tl in tok_lists))
    blocks = make_blocks(cap)

    xT = np.ascontiguousarray(xf.T).astype(BF16_NP)  # [H, T] bf16
    wrT_p = _pack_kpf(
        np.ascontiguousarray(np.asarray(Wr, np.float32).T).astype(BF16_NP), KH
    )
    bi_full = np.asarray(bi, np.float32)
    bo_full = np.asarray(bo, np.float32)

    in_maps = []
    for c in range(N_CORES):
        tl = tok_lists[c]
        xg = np.zeros((H, cap), dtype=BF16_NP)
        xg[:, : len(tl)] = xT[:, tl]
        # block-major packing: [128, sum_b KH*b], block ib at offset KH*t0
        xg_k = xg.reshape(KH, P, cap)
        xg_p = np.empty((P, KH * cap), dtype=BF16_NP)
        t0 = 0
        for b in blocks:
            xg_p[:, KH * t0 : KH * (t0 + b)] = (
                xg_k[:, :, t0 : t0 + b].transpose(1, 0, 2).reshape(P, KH * b)
            )
            t0 += b
        sel = np.zeros(E, np.float32)
        sel[c] = 1.0
        in_maps.append(
            {
                "xg": xg_p,
                "wiT": _pack_wi_groups(
                    np.asarray(Wi[c], np.float32).T.astype(BF16_NP)
                ),
                "woT": _pack_kpf(
                    np.ascontiguousarray(np.asarray(Wo[c], np.float32).T).astype(
                        BF16_NP
                    ),
                    KF,
                ),
                "wrT": wrT_p,
                "bi": _pack_kpf(bi_full[c].reshape(F, 1), KF).reshape(P, KF),
                "bo": bo_full[c],
                "esel": sel,
            }
        )

    _ensure_axon_hooks_module()
    nc = _get_nc(cap)
    res = run_bass_kernel_spmd(
        nc, in_maps, core_ids=list(range(N_CORES)), trace=_trace
    )

    # Unshard: scatter-add the per-expert (already routing-weighted) rows.
    out = np.zeros((T, H), dtype=np.float32)
    for c in range(N_CORES):
        tl = tok_lists[c]
        # out param is [128, ntile_total, H]: token t0+ts*128+p -> [p, n0+ts, :]
        o = res.results[c]["out"]  # [P, NT, H]
        o_rows = o.transpose(1, 0, 2).reshape(-1, H)[: len(tl)]
        out[tl] += o_rows
    out = out.reshape(x.shape)
    if _trace:
        return out, res
    return out
